# revision 17
# baseline (speedup 1.0000x reference)
"""Trainium2 Bass kernel for nn_DihedralAngleLayer.

Input:  x [2_000_000, 42] f32 (14 atoms x 3 coords per row),
        mask_matrix [4, 14] f32 one-hot carbon selector.
Output: dihedral angle per row, [2_000_000] f32.

Data-parallel across 8 NeuronCores: rows are padded to 8*250_112 and split
evenly. Each core owns rows in global partition-major order: partition p
handles rows [p*Q, (p+1)*Q), Q = rows/128.

Per tile (G rows of every partition) the Vector engine computes

    a = c0-c1, b = c2-c1, d = c3-c2, m = d x b  (3 split-AP multiplies, no
    duplicated subs), prods a*(b|d|m) and b*d, dots via two 5-plane strided
    adds (cheaper than the 1x-mode reduce), then
    xx = p*q - r*s, t2 = sqrt(q)*det / xx  (reciprocal_approx_fast).

The tail is a signed-ratio arctan: out = arctan(t2) + sign(t2)*(-pi*[xx<0]),
so per row only arctan + 2 tensor ops remain.  ScalarE ops are split so
heads touch only sqrt_and_others table funcs (Square/Sqrt) and tails only
sigmoid_and_others funcs (Arctan/Sign/Copy), bounding ACT table reloads to
two per chunk.  DVE model: (N+151)/0.96 ns per op -> ~46 elems/row + 15
instrs/tile; big G amortizes the per-instruction overhead.
"""

import numpy as np

import concourse.bacc as bacc
import concourse.bass as bass
import concourse.dve_ops as dve_ops_mod
import concourse.mybir as mybir
from concourse.bass_utils import run_bass_kernel_spmd
from concourse.dve_spec import C0, C1, AluOp, Bin, Spec, Src0, Src1, lower
from concourse.dve_uop import DveOpSpec
from concourse.tile import TileContext

AF = mybir.ActivationFunctionType
OP = mybir.AluOpType
F32 = mybir.dt.float32


def _register_recip1_mul():
    """Custom DVE op: out = in1 * approx(1/in0) (BITWISE_NOT seed + 1 NR pass,
    ~0.17% max rel err -> <1e-3 rad angle error, vs the 2e-2 gate).  Fuses the
    reciprocal and the t2 multiply into one 6-stage instruction."""
    name = "RECIP1_MUL_ANT"
    for op in dve_ops_mod.OPS:
        if op.name == name:
            return op
    not_x = Bin(AluOp.BITWISE_NOT, Src0, Src0)
    y0 = not_x * C0
    y1 = y0 * (C1 - Src0 * y0)

    def _ref(in0, in1, c0, c1, c2):
        nx = (~in0.view(np.int32)).view(np.float32)
        r0 = nx * c0
        r1 = r0 * (c1 - in0 * r0)
        return (in1 * r1).astype(np.float32)

    spec = Spec(body=Src1 * y1, reference=_ref)
    shas = {
        ver: DveOpSpec(name=name, opcode=0, uops=lower(spec, ver=ver), rd1_en=True).sha(ver)
        for ver in ("v3", "v4")
    }
    op = dve_ops_mod.DveOp(name, spec, subdim=False, uops_sha=shas)
    dve_ops_mod.OPS.append(op)
    row = dve_ops_mod._CUSTOM_DVE_ROW_BASE + len(dve_ops_mod.OPS) - 1
    assert row < 0x20, "custom-DVE opcode rows exhausted"
    dve_ops_mod._SUB_OPCODE_FOR_NAME[name] = row
    dve_ops_mod.CUSTOM_DVE_SPECS[name] = spec
    return op


RECIP1_MUL = _register_recip1_mul()
_RC = dve_ops_mod.RECIP_APPROX_FAST_CONSTS

PI = float(np.pi)

N_CORES = 8
# geometric ramp: DVE starts on the first small tile and never starves while
# the (faster) DMA stream builds its lead; small last chunk bounds end drain
TILES = [64, 96, 128, 192, 256, 256, 256, 256, 256, 194]
CHUNK_AFTER = {4, 7, 8}              # tail chunks close after these + last
Q = sum(TILES)                       # rows per partition (1954)
ROWS_PER_CORE = 128 * Q              # 250_112
CHUNK_MAX = 768

# per-row scratch layout (period PER floats)
# a@0 b@3 d@6 m@9 n@12 P1@15 P2@18 P1n@21 P2n@24 prods@27 (det,xx,q x3)
PER = 36
S_A, S_B, S_D, S_M, S_N = 0, 3, 6, 9, 12
S_P1, S_P2, S_P1N, S_P2N, S_PR = 15, 18, 21, 24, 27
# mini planes ([G] each): det xx q sq yy
M_DET, M_XX, M_Q, M_SQ, M_YY = range(5)


def _ap(base, off, dims):
    return bass.AP(
        base.tensor, base.offset + off, [list(base.ap[0])] + [list(d) for d in dims]
    )


def _emit_head(nc, xp, scp, mp, x, planes, toff, G, c0, c1, c2, c3):
    """Per-tile head: subs, cross, dots, xx/t2 -> full-length planes."""
    v, s = nc.vector, nc.scalar

    xt = xp.tile([128, G * 42], F32, tag="x")
    sc = scp.tile([128, G * PER], F32, tag="sc")
    mi = mp.tile([128, G * 5], F32, tag="mi")

    nc.sync.dma_start(
        out=xt[:],
        in_=x.rearrange("(p q) c -> p q c", p=128)[:, toff : toff + G, :],
    )

    xa, sa, ma = xt[:], sc[:], mi[:]

    def xap(off, dims):
        return _ap(xa, off, [[42, G]] + dims)

    def sap(off, dims=()):
        return _ap(sa, off, [[PER, G]] + list(dims))

    def map_(k, dims=None):
        return _ap(ma, k * G, dims if dims is not None else [[1, G]])

    # a = c0-c1 and b = c2-c1 fused (in0 strides over {c0,c2}, in1 reads c1 twice)
    v.tensor_tensor(
        sap(S_A, [[3, 2], [1, 3]]),
        xap(c0, [[c2 - c0, 2], [1, 3]]),
        xap(c1, [[0, 2], [1, 3]]),
        OP.subtract,
    )
    # d = c3-c2
    v.tensor_tensor(sap(S_D, [[1, 3]]), xap(c3, [[1, 3]]), xap(c2, [[1, 3]]), OP.subtract)
    # q prods = b*b on ScalarE (Square; sqrt_and_others set), overlaps DVE below
    s.activation(sap(S_PR + 6, [[1, 3]]), sap(S_B, [[1, 3]]), AF.Square)
    # Lagrange form: xx = (a x b).(d x b), det = a.(d x b) -- two cross
    # products, computed by three pairwise-merged split-AP multiplies.
    # m = d x b = P2-P1, n = a x b = P1n-P2n.
    # P1x: (P1[0],P1[1]) = (by,bz)*(dz,dx) ; (P1n[0],P1n[1]) = (ay,az)*(bz,bx)
    v.tensor_tensor(
        sap(S_P1, [[6, 2], [1, 2]]),
        sap(S_B + 1, [[-3, 2], [1, 2]]),
        sap(S_D + 2, [[-3, 2], [-2, 2]]),
        OP.mult,
    )
    # P2x: (P2[0],P2[1]) = (bz,bx)*(dy,dz) ; (P2n[0],P2n[1]) = (az,ax)*(by,bz)
    v.tensor_tensor(
        sap(S_P2, [[6, 2], [1, 2]]),
        sap(S_B + 2, [[-3, 2], [-2, 2]]),
        sap(S_D + 1, [[-3, 2], [1, 2]]),
        OP.mult,
    )
    # Pcx: (P1[2],P2[2]) = (bx,by)*(dy,dx) ; (P1n[2],P2n[2]) = (ax,ay)*(by,bx)
    v.tensor_tensor(
        sap(S_P1 + 2, [[6, 2], [3, 2]]),
        sap(S_B, [[-3, 2], [1, 2]]),
        sap(S_D + 1, [[-3, 2], [-1, 2]]),
        OP.mult,
    )
    # m = P2-P1, n = P1n-P2n in one op
    v.tensor_tensor(
        sap(S_M, [[3, 2], [1, 3]]),
        sap(S_P2, [[3, 2], [1, 3]]),
        sap(S_P1, [[9, 2], [1, 3]]),
        OP.subtract,
    )
    # prods (a*m -> det terms, n*m -> xx terms) in one op
    v.tensor_tensor(
        sap(S_PR, [[3, 2], [1, 3]]),
        sap(S_A, [[12, 2], [1, 3]]),
        sap(S_M, [[0, 2], [1, 3]]),
        OP.mult,
    )
    # segmented reduce -> (det, xx, q) mini-planes (unit inner stride)
    v.reduce_sum(
        map_(M_DET, [[G, 3], [1, G]]),
        _ap(sa, S_PR, [[3, 3], [PER, G], [1, 3]]),
        axis=mybir.AxisListType.X,
    )
    # xx -> full plane for the tail's sign-of-xx correction (ScalarE Copy,
    # present in every ACT table set)
    s.activation(_ap(planes, toff, [[1, G]]), map_(M_XX), AF.Copy)
    s.activation(map_(M_SQ), map_(M_Q), AF.Sqrt)
    # yy = sq*det, then fused t2 = yy * approx(1/xx) -> full plane
    v.tensor_tensor(map_(M_YY), map_(M_SQ), map_(M_DET), OP.mult)
    v._custom_dve(
        RECIP1_MUL,
        out=_ap(planes, Q + toff, [[1, G]]),
        in0=map_(M_XX),
        in1=map_(M_YY),
        s0=_RC["s0"],
        s1=_RC["s1"],
    )


def _emit_tail_scalar(nc, tsp, planes, toff, FD):
    """Chunk tail, ScalarE half: al2 = arctan(t2), s2 = sign(t2),
    cpl = -pi*[xx<0].  All funcs live in sigmoid_and_others (one table set).
    Returns the scratch tile for the deferred Vector half."""
    s = nc.scalar
    ts = tsp.tile([128, 3 * CHUNK_MAX], F32, tag="ts")
    ta = ts[:]

    def cap(k):
        return _ap(ta, k * CHUNK_MAX, [[1, FD]])

    s.activation(cap(0), _ap(planes, Q + toff, [[1, FD]]), AF.Arctan)   # al2
    s.activation(cap(1), _ap(planes, Q + toff, [[1, FD]]), AF.Sign)     # s2
    s.activation(cap(2), _ap(planes, toff, [[1, FD]]), AF.Sign, scale=-1.0)
    s.activation(cap(2), cap(2), AF.Copy, scale=-PI / 2, bias=-PI / 2)  # cpl
    return ts


def _emit_tail_vector(nc, ts, outp, y, toff, FD):
    """Chunk tail, Vector half: out = al2 + s2*cpl.  Emitted AFTER the next
    head's ops so the in-order DVE queue never stalls on the ScalarE chain."""
    v = nc.vector
    ot = outp.tile([128, CHUNK_MAX], F32, tag="o")
    ta = ts[:]

    def cap(k):
        return _ap(ta, k * CHUNK_MAX, [[1, FD]])

    v.tensor_tensor(cap(2), cap(1), cap(2), OP.mult)              # c = s2*cpl
    v.tensor_tensor(_ap(ot[:], 0, [[1, FD]]), cap(0), cap(2), OP.add)
    nc.sync.dma_start(
        out=y.rearrange("(p q) -> p q", p=128)[:, toff : toff + FD],
        in_=_ap(ot[:], 0, [[1, FD]]),
    )


def build_kernel(atoms):
    c0, c1, c2, c3 = (3 * int(a) for a in atoms)
    nc = bacc.Bacc("TRN2", target_bir_lowering=False, debug=False)
    x = nc.dram_tensor("x", [ROWS_PER_CORE, 42], F32, kind="ExternalInput")
    y = nc.dram_tensor("y", [ROWS_PER_CORE], F32, kind="ExternalOutput")
    with TileContext(nc) as tc:
        with (
            tc.tile_pool(name="xp", bufs=2) as xp,
            tc.tile_pool(name="scp", bufs=1) as scp,
            tc.tile_pool(name="mp", bufs=1) as mp,
            tc.tile_pool(name="plp", bufs=1) as plp,
            tc.tile_pool(name="tsp", bufs=2) as tsp,
            tc.tile_pool(name="outp", bufs=2) as outp,
        ):
            pl_tile = plp.tile([128, 2 * Q], F32, tag="pl")
            planes = pl_tile[:]
            # Tail emission is deferred: the ScalarE half goes out one head
            # after the chunk closes, the Vector half two heads after, so
            # neither the in-order DVE queue nor the ACT-table reloads ever
            # stall the per-tile pipeline.
            toff = 0
            chunk_start = 0
            chunks = []
            for i, G in enumerate(TILES):
                _emit_head(nc, xp, scp, mp, x, planes, toff, G, c0, c1, c2, c3)
                for ch in chunks:
                    if ch["ts"] is not None and not ch["v"] and ch["at"] <= i - 2:
                        _emit_tail_vector(nc, ch["ts"], outp, y, ch["o"], ch["n"])
                        ch["v"] = True
                for ch in chunks:
                    if ch["ts"] is None and ch["at"] <= i - 1:
                        ch["ts"] = _emit_tail_scalar(nc, tsp, planes, ch["o"], ch["n"])
                toff += G
                if i in CHUNK_AFTER or i == len(TILES) - 1:
                    chunks.append(
                        {"at": i, "o": chunk_start, "n": toff - chunk_start,
                         "ts": None, "v": False}
                    )
                    chunk_start = toff
            for ch in chunks:
                if ch["ts"] is None:
                    ch["ts"] = _emit_tail_scalar(nc, tsp, planes, ch["o"], ch["n"])
            for ch in chunks:
                if not ch["v"]:
                    _emit_tail_vector(nc, ch["ts"], outp, y, ch["o"], ch["n"])
    nc.finalize()
    return nc


_CACHE = {}


def _get_nc(atoms):
    key = tuple(int(a) for a in atoms)
    if key not in _CACHE:
        _CACHE[key] = build_kernel(key)
    return _CACHE[key]


def run(x, atoms=(0, 4, 7, 11), **spmd_kwargs):
    """x: [B, 42] f32. Returns (y [B] f32, BassKernelResults)."""
    x = np.ascontiguousarray(np.asarray(x, dtype=np.float32))
    B = x.shape[0]
    total = N_CORES * ROWS_PER_CORE
    if B < total:
        # pad with replicated leading rows (valid, non-degenerate data)
        x = np.concatenate([x, x[: total - B]], axis=0)
    nc = _get_nc(atoms)
    shards = x.reshape(N_CORES, ROWS_PER_CORE, 42)
    in_maps = [{"x": shards[i]} for i in range(N_CORES)]
    res = run_bass_kernel_spmd(nc, in_maps, core_ids=list(range(N_CORES)), **spmd_kwargs)
    y = np.concatenate([r["y"] for r in res.results])[:B]
    return np.asarray(y, dtype=np.float32), res


def kernel(x, mask_matrix):
    mask = np.asarray(mask_matrix)
    atoms = tuple(int(i) for i in np.argmax(mask, axis=1))
    y, _ = run(x, atoms=atoms)
    return y
